# revision 4
# baseline (speedup 1.0000x reference)
"""Locally-connected conv (per-location weights) + ReLU on 8 Trainium2 cores.

Problem: x (B=64, Cin=64, H=64, W=64), weights (H, W, Cout=64, Cin=64, 3, 3)
  out[r,a,i,j] = relu( sum_{b,c,d} weights[i,j,a,b,c,d] * xpad[r,b,i+c,j+d] )

Sharding: data-parallel over H - core cid owns output rows i in [8*cid, 8*cid+8).

v2 vs the 250us baseline:
  * K=128 tap pairing: taps c=0,1 of each output row contract in ONE matmul
    (x planes p, p+1 stacked in the 128 partitions), tap c=2 stays K=64.
    2 matmuls per (row, v) instead of 3 -> tensor-engine column streams drop
    by 1/3.  All stationaries sit at PE row position 0; output row parity
    selects the PE column group (out partitions 0-63 / 64-127), so even/odd
    rows stream concurrently.
  * x deduplication: HBM carries each padded x plane once ([64, 66, 64] per
    plane); the (p, p+1) partition stacking is built on-chip by an
    SBUF->SBUF DMA (plane p+1 lower half -> plane p tile upper half).
    Saves ~4.3 MB/core of HBM reads vs the duplicated layout.
  * Per-plane x tiles in a 6-deep ring: matmuls start after ~2 plane loads
    instead of after the whole 9.7 MB x block.
  * Drains split: ScalarE drains pslo while DVE drains pshi.

PSUM: row 2t accumulates [r, (j, a)] on partitions 0-63 over all 8 banks
(pslo = j<32, pshi = j>=32), row 2t+1 on partitions 64-127.  Each bank is
zero-initialized once per row pair by a full-bank matmul with zero weights
(start=True), so every real matmul just accumulates (start=False) and may
span any column window within a bank.  skip_group_check: the sim's global
group tracker mis-addresses base-partition-64 PSUM APs (its per-tensor
pending-zero model, which is what models HW, is correct).

Weights stream as bf16: fp8 e3m4 storage crashes the PE exec unit
(NRT_EXEC_UNIT_UNRECOVERABLE, found empirically - only e4m3/e5m2 stream),
and e4m3's 3-bit mantissa pushes max rel err to 2.8e-2 > the 2e-2 gate.
"""

import ml_dtypes
import numpy as np

import concourse.bass as bass
import concourse.mybir as mybir
import concourse.tile as tile
from concourse import bacc
from concourse.bass_utils import run_bass_kernel_spmd

B = 64          # batch (= stationary M / out free dim per matmul)
CIN = 64        # in channels
COUT = 64       # out channels
H = 64
W = 64
KS = 3
NCORES = 8
RPC = H // NCORES        # output rows per core = 8
NT = RPC // 2            # row pairs per core = 4
NV = W + 2               # padded columns = 66
NPL = RPC + 2            # x planes per core = 10
QB = (0, 17, 33, 50, 66)  # v-quarter boundaries
NQ = 4
CHQ = 17 * 3 * COUT      # max weight cols per quarter-chunk = 3264
FP32 = mybir.dt.float32

WDT = mybir.dt.bfloat16
NP_WDT = ml_dtypes.bfloat16
XDT = mybir.dt.bfloat16
NP_XDT = ml_dtypes.bfloat16
ODT = mybir.dt.bfloat16

_PROGRAM = None
LAST_RESULTS = None


def _segments(v):
    """PSUM j-window for input column v, split at bank (8-j) boundaries."""
    jlo, jhi = max(0, v - 2), min(W - 1, v)
    if jlo // 8 == jhi // 8:
        return [(jlo, jhi)]
    mid = 8 * (jhi // 8)
    return [(jlo, mid - 1), (mid, jhi)]


def _build_program():
    nc = bacc.Bacc("TRN2", target_bir_lowering=False, debug=False,
                   num_devices=NCORES)
    # xt[b, p, v, r]: padded x plane u0+p for channel b (planes stored once).
    xt = nc.dram_tensor("xt", [64, NPL, NV, B], XDT, kind="ExternalInput")
    # wp01[i, q, k, col]: k<64 = tap c=0 weights over b, k>=64 = tap c=1;
    # col = (vv*3 + m)*64 + a with j = v-2+m, d = 2-m (invalid j -> zero
    # columns, never streamed).
    wp01 = nc.dram_tensor("wp01", [RPC, NQ, 128, CHQ], WDT,
                          kind="ExternalInput")
    # wp2[i, q, b, col]: tap c=2 weights.
    wp2 = nc.dram_tensor("wp2", [RPC, NQ, 64, CHQ], WDT,
                         kind="ExternalInput")
    # ot[t, jhalf, (parity, r), (j%32, a)]
    ot = nc.dram_tensor("ot", [NT, 2, 128, 2048], ODT, kind="ExternalOutput")

    with tile.TileContext(nc) as tc:
        with (
            tc.tile_pool(name="xpool", bufs=6) as xpool,
            tc.tile_pool(name="w01pool", bufs=6) as w01pool,
            tc.tile_pool(name="w2pool", bufs=6) as w2pool,
            tc.tile_pool(name="opool", bufs=4) as opool,
            tc.tile_pool(name="zpool", bufs=1) as zpool,
            tc.tile_pool(name="pspool", bufs=1,
                         space=bass.MemorySpace.PSUM) as pspool,
        ):
            zt = zpool.tile([64, 512], XDT, tag="zt")
            nc.vector.memset(zt[:], 0.0)

            # xtiles[p][0:64] = plane u0+p, xtiles[p][64:128] = plane u0+p+1
            # (filled when plane p+1 loads).
            xtiles = {}

            def load_plane(p):
                xq = xpool.tile([128, NV, B], XDT, tag="xp")
                nc.sync.dma_start(xq[0:64], xt[:, p])
                if p > 0:
                    nc.sync.dma_start(xtiles[p - 1][64:128], xq[0:64])
                xtiles[p] = xq

            for p in range(6):
                load_plane(p)

            for t in range(NT):
                pslo = pspool.tile([128, 2048], FP32, tag="pslo")
                pshi = pspool.tile([128, 2048], FP32, tag="pshi")
                for k in range(4):
                    nc.tensor.matmul(pslo[:, 512 * k:512 * (k + 1)],
                                     zt[:, 0:128], zt[:, 0:512],
                                     start=True, stop=False,
                                     skip_group_check=True)
                    nc.tensor.matmul(pshi[:, 512 * k:512 * (k + 1)],
                                     zt[:, 0:128], zt[:, 0:512],
                                     start=True, stop=False,
                                     skip_group_check=True)
                for q in range(NQ):
                    ncols = (QB[q + 1] - QB[q]) * 3 * COUT
                    wE01 = w01pool.tile([128, CHQ], WDT, tag="w01")
                    wO01 = w01pool.tile([128, CHQ], WDT, tag="w01")
                    wE2 = w2pool.tile([64, CHQ], WDT, tag="w2")
                    wO2 = w2pool.tile([64, CHQ], WDT, tag="w2")
                    nc.sync.dma_start(wE01[:, :ncols], wp01[2 * t, q, :, :ncols])
                    nc.sync.dma_start(wO01[:, :ncols],
                                      wp01[2 * t + 1, q, :, :ncols])
                    nc.sync.dma_start(wE2[:, :ncols], wp2[2 * t, q, :, :ncols])
                    nc.sync.dma_start(wO2[:, :ncols],
                                      wp2[2 * t + 1, q, :, :ncols])
                    for vv in range(QB[q + 1] - QB[q]):
                        v = QB[q] + vv
                        segs = _segments(v)
                        groups = (
                            (xtiles[2 * t][0:128, v, :], wE01, 0),
                            (xtiles[2 * t + 1][0:128, v, :], wO01, 64),
                            (xtiles[2 * t + 2][0:64, v, :], wE2, 0),
                            (xtiles[2 * t + 3][0:64, v, :], wO2, 64),
                        )
                        for lhsT, wt, pb in groups:
                            for (j0, j1) in segs:
                                m0 = j0 - v + 2
                                n = (j1 - j0 + 1) * 64
                                tgt = pslo if j0 < 32 else pshi
                                c0 = (j0 % 32) * 64
                                w0 = (vv * 3 + m0) * 64
                                nc.tensor.matmul(
                                    tgt[pb:pb + 64, c0:c0 + n],
                                    lhsT, wt[:, w0:w0 + n],
                                    start=False, stop=False,
                                    skip_group_check=True)
                for p in (2 * t + 6, 2 * t + 7):
                    if p < NPL:
                        load_plane(p)
                olo = opool.tile([128, 2048], ODT, tag="olo")
                ohi = opool.tile([128, 2048], ODT, tag="ohi")
                nc.scalar.activation(olo[:], pslo[:],
                                     mybir.ActivationFunctionType.Relu)
                nc.vector.tensor_relu(ohi[:], pshi[:])
                nc.sync.dma_start(ot[t, 0], olo[:])
                nc.sync.dma_start(ot[t, 1], ohi[:])
    nc.compile()
    return nc


def _pack_weights(weights):
    """weights (i, j, a, b, c, d) fp32 -> F[i, c, b, v, m, a] with
    F[i,c,b,v,m,a] = W[i, v-2+m, a, b, c, 2-m] (zero at invalid j)."""
    T2 = weights.transpose(0, 4, 5, 2, 3, 1)[:, :, ::-1]  # [i, c, m, a, b, j]
    T2p = np.ascontiguousarray(np.pad(T2, [(0, 0)] * 5 + [(2, 2)]))
    s = T2p.strides
    E = np.lib.stride_tricks.as_strided(
        T2p, shape=(H, KS, NV, KS, COUT, CIN),
        strides=(s[0], s[1], s[5], s[2] + s[5], s[3], s[4]))
    # E[i, c, v, m, a, b] -> F[i, c, b, v, m, a]
    return E.transpose(0, 1, 5, 2, 3, 4).astype(NP_WDT)


def _prep_x(x):
    xpad = np.pad(x, ((0, 0), (0, 0), (1, 1), (1, 1)))
    return np.ascontiguousarray(xpad.transpose(1, 2, 3, 0)).astype(NP_XDT)


def _core_inputs(F, xf, cid):
    u0 = RPC * cid
    xt_core = np.ascontiguousarray(xf[:, u0:u0 + NPL])  # [64, 10, 66, 64]
    Fc = F[u0:u0 + RPC]                                 # [8, 3, 64, 66, 3, 64]
    wp01 = np.zeros((RPC, NQ, 128, CHQ), dtype=NP_WDT)
    wp2 = np.zeros((RPC, NQ, 64, CHQ), dtype=NP_WDT)
    for i in range(RPC):
        W01 = np.concatenate([Fc[i, 0], Fc[i, 1]], axis=0).reshape(128, -1)
        W2 = Fc[i, 2].reshape(64, -1)
        for q in range(NQ):
            c0, c1 = QB[q] * 192, QB[q + 1] * 192
            wp01[i, q, :, :c1 - c0] = W01[:, c0:c1]
            wp2[i, q, :, :c1 - c0] = W2[:, c0:c1]
    return {"xt": xt_core, "wp01": wp01, "wp2": wp2}


def _unpack_out(o):
    o = np.asarray(o)                       # [4, 2, 128, 2048] bf16
    o = o.reshape(NT, 2, 2, B, 32, COUT)    # [t, jh, par, r, jr, a]
    o = o.transpose(3, 5, 0, 2, 1, 4)       # [r, a, t, par, jh, jr]
    return o.reshape(B, COUT, RPC, W)


def kernel(x, weights):
    global _PROGRAM, LAST_RESULTS
    x = np.ascontiguousarray(np.asarray(x, dtype=np.float32))
    weights = np.ascontiguousarray(np.asarray(weights, dtype=np.float32))
    assert x.shape == (B, CIN, H, W) and weights.shape == (H, W, COUT, CIN, KS, KS)

    F = _pack_weights(weights)      # [64, 3, 64, 66, 3, 64]
    xf = _prep_x(x)                 # [b, u, v, r] bf16, u in [0, 66)

    in_maps = [_core_inputs(F, xf, cid) for cid in range(NCORES)]

    if _PROGRAM is None:
        _PROGRAM = _build_program()
    res = run_bass_kernel_spmd(_PROGRAM, in_maps, list(range(NCORES)))
    LAST_RESULTS = res

    outs = [_unpack_out(res.results[c]["ot"]) for c in range(NCORES)]
    full = np.concatenate(outs, axis=2).astype(np.float32)
    return np.ascontiguousarray(full)
